# revision 19
# baseline (speedup 1.0000x reference)
"""Trainium2 Bass kernel for causal GQA attention (B=2, T=2048, E=2048, H=16, D=128, KVH=8).

Sharding: 8 cores = 2 (batch) x 4 (head groups). Each core computes 4 query heads
(column-parallel wq) + their 2 KV heads, full causal attention for those heads, and
a partial output projection (row-parallel wo). Host sums the 4 partials per batch.

Layout strategy: everything head-transposed ([D, T] with D on partitions) so that
no on-chip transposes are needed anywhere:
  - qT/kT = wq/wk.T @ x.T directly from PE (lhsT = weight slice, rhs = x.T)
  - scores S^T[k, q] = kT.T @ qT (lhsT = kT block, rhs = qT chunk)
  - attn_outT[d, q] = v_nat.T @ expS^T (lhsT = v natural [t, d], rhs = exp block)
  - out[t, e] = attn_outT.T @ wo (lhsT = attn_outT, rhs = wo rows)
RoPE pairs are de-interleaved by permuting wq/wk columns on the host (scores are
invariant since q and k use the same permutation), so rope becomes a half-swap.
Softmax is computed without max-subtraction (logits are O(5)); causal masking is a
0/1 multiply on the exp'd diagonal blocks, full blocks above the diagonal skipped.
Denominators via ones-vector matmuls accumulated in PSUM alongside the AV matmuls.

`reps`: wraps the whole body (including input DMA) in an on-device For_i loop —
used only for latency-slope timing in test.py; the graded path uses reps=1.
"""

import numpy as np
import ml_dtypes

BF16 = ml_dtypes.bfloat16

B, T, E = 2, 2048, 2048
H, D = 16, 128
KVH = 8
THETA = 10000.0
P = 128
EO = E // P          # 16 contraction chunks
CH = 512             # q-chunk width
NTQ = T // CH        # 4 q chunks
NTB = T // P         # 16 t blocks
NH = H // 4          # 4 q heads per core
NKV = 2              # kv heads per core
SCALE = float(D) ** -0.5

_NC_CACHE = {}
_PHASE_LIMIT = "full"   # "proj" | "attn" | "full" — for phase-cost probing only


def _build_nc(reps=1):
    import concourse.mybir as mybir
    import concourse.tile as tile
    from concourse import bacc

    nc = bacc.Bacc(None, target_bir_lowering=False)
    dt = mybir.dt
    f32, bf16 = dt.float32, dt.bfloat16
    Exp = mybir.ActivationFunctionType.Exp

    xT_d = nc.dram_tensor("xT", [E, T], bf16, kind="ExternalInput")
    wq_d = nc.dram_tensor("wq", [E, NH * D], bf16, kind="ExternalInput")
    wk_d = nc.dram_tensor("wk", [E, NKV * D], bf16, kind="ExternalInput")
    wv_d = nc.dram_tensor("wv", [E, NKV * D], bf16, kind="ExternalInput")
    wo_d = nc.dram_tensor("wo", [NH * D, E], bf16, kind="ExternalInput")
    cos_d = nc.dram_tensor("cosd", [P, T], f32, kind="ExternalInput")
    sin_d = nc.dram_tensor("sind", [P, T], f32, kind="ExternalInput")
    mk_d = nc.dram_tensor("mkd", [4, P, CH], bf16, kind="ExternalInput")
    o_d = nc.dram_tensor("od", [T, E], bf16, kind="ExternalOutput")

    xT_r = xT_d.rearrange("(eo p) t -> p eo t", p=P)
    wq_r = wq_d.rearrange("(eo p) m -> p eo m", p=P)
    wk_r = wk_d.rearrange("(eo p) m -> p eo m", p=P)
    wv_r = wv_d.rearrange("(eo p) m -> p eo m", p=P)
    wo_r = wo_d.rearrange("(h p) e -> p h e", p=P)
    o_r = o_d.rearrange("(tb p) e -> p tb e", p=P)

    with tile.TileContext(nc) as tc:
        with (
            tc.tile_pool(name="singles", bufs=1) as sg,
            tc.tile_pool(name="ropet", bufs=2) as rp,
            tc.tile_pool(name="expp", bufs=6) as ep,
            tc.tile_pool(name="normp", bufs=2) as np_,
            tc.tile_pool(name="outst", bufs=2) as op_,
        ):

            def emit_body():
                # tiles (allocation only; DMA issue order below is what matters)
                wk_sb = sg.tile([P, EO, NKV * D], bf16, name="wk_sb", tag="wk_sb")
                xT_sb = sg.tile([P, EO, T], bf16, name="xT_sb", tag="xT_sb")
                wv_sb = sg.tile([P, EO, NKV * D], bf16, name="wv_sb", tag="wv_sb")
                wq_sb = sg.tile([P, EO, NH * D], bf16, name="wq_sb", tag="wq_sb")
                cos_sb = sg.tile([P, T], f32, name="cos_sb", tag="cos_sb")
                sin_sb = sg.tile([P, T], f32, name="sin_sb", tag="sin_sb")
                wo_sb = sg.tile([P, NH, E], bf16, name="wo_sb", tag="wo_sb")
                mk_sb = [sg.tile([P, CH], bf16, name=f"mk{i}", tag=f"mk{i}") for i in range(4)]

                # DMA issue order = consumption order
                nc.sync.dma_start(wk_sb[:], wk_r[:])
                for eo in range(EO):
                    nc.sync.dma_start(xT_sb[:, eo, :], xT_r[:, eo, :])
                nc.sync.dma_start(cos_sb[:], cos_d[:])
                nc.sync.dma_start(sin_sb[:], sin_d[:])
                nc.sync.dma_start(wv_sb[:], wv_r[:])
                nc.sync.dma_start(wq_sb[:], wq_r[:])
                for i in range(4):
                    nc.sync.dma_start(mk_sb[i][:], mk_d[i])
                nc.sync.dma_start(wo_sb[:], wo_r[:])

                ones_sb = sg.tile([P, 1], bf16, name="ones_sb", tag="ones_sb")
                nc.vector.memset(ones_sb[:], 1.0)

                kT_sb = [sg.tile([P, T], bf16, name=f"kT{g}", tag=f"kT{g}") for g in range(NKV)]
                qT_sb = [sg.tile([P, T], bf16, name=f"qT{h}", tag=f"qT{h}") for h in range(NH)]
                v_sb = sg.tile([P, NTB, NKV * D], bf16, name="v_sb", tag="v_sb")

                def rope_chunk(dest, sl, ps):
                    # dest[:, sl] = ps * cos + swap_halves(ps) * sin (sin rows 0:64 pre-negated)
                    t1 = rp.tile([P, CH], f32, name="ropet1", tag="ropet1")
                    nc.vector.tensor_mul(t1[:], ps[:], cos_sb[:, sl])
                    t2 = rp.tile([P, CH], f32, name="ropet2", tag="ropet2")
                    nc.vector.tensor_mul(t2[0:64, :], ps[64:128, :], sin_sb[0:64, sl])
                    nc.vector.tensor_mul(t2[64:128, :], ps[0:64, :], sin_sb[64:128, sl])
                    nc.vector.tensor_add(dest[:, sl], t1[:], t2[:])

                with (
                    tc.tile_pool(name="pj", bufs=3, space="PSUM") as pj,
                    tc.tile_pool(name="ps_s", bufs=2, space="PSUM") as ps_s,
                    tc.tile_pool(name="ps_o", bufs=2, space="PSUM") as ps_o,
                    tc.tile_pool(name="ps_m", bufs=1, space="PSUM") as ps_m,
                ):
                    def proj_qk(w_sb, col, dest, eo_outer=False):
                        # chunk-major, eo-inner: consecutive matmuls accumulate into the
                        # same PSUM bank (avoids the HW bank-cycling HAM penalty).
                        # eo_outer=True (first unit only): consume xT chunks as the input
                        # DMA delivers them — PE is DMA-starved there anyway.
                        if eo_outer:
                            groups = ((0, 1, 2), (3,))
                        else:
                            groups = ((0,), (1,), (2,), (3,))
                        for chunks in groups:
                            psqs = {tci: pj.tile([P, CH], f32, name=f"psq{tci}", tag="pj")
                                    for tci in chunks}
                            for eo in range(EO):
                                for tci in chunks:
                                    nc.tensor.matmul(
                                        psqs[tci][:],
                                        w_sb[:, eo, D * col:D * (col + 1)],
                                        xT_sb[:, eo, CH * tci:CH * (tci + 1)],
                                        start=(eo == 0), stop=(eo == EO - 1),
                                    )
                            for tci in chunks:
                                rope_chunk(dest, slice(CH * tci, CH * (tci + 1)), psqs[tci])

                    def attn(h, tci):
                        g = h // 2
                        sl = slice(CH * tci, CH * (tci + 1))
                        ntk = 4 * tci + 4
                        o_ps = ps_o.tile([P, CH], f32, name="o_ps", tag="o_ps")
                        s_row = ps_m.tile([1, CH], f32, name="s_row", tag="s_row")
                        for j in range(ntk):
                            # diagonal blocks only need columns >= 128*di (rest fully masked)
                            di = j - 4 * tci
                            c0 = P * di if di > 0 else 0
                            qsl = slice(CH * tci + c0, CH * (tci + 1))
                            s_ps = ps_s.tile([P, CH], f32, name="s_ps", tag="s_ps")
                            nc.tensor.matmul(
                                s_ps[:, c0:], kT_sb[g][:, P * j:P * (j + 1)], qT_sb[h][:, qsl],
                                start=True, stop=True,
                            )
                            e_t = ep.tile([P, CH], bf16, name="e_t", tag="e_t")
                            nc.scalar.activation(e_t[:, c0:], s_ps[:, c0:], Exp, scale=SCALE)
                            if di >= 0:
                                nc.vector.tensor_mul(e_t[:, c0:], e_t[:, c0:], mk_sb[di][:, c0:])
                            nc.tensor.matmul(
                                o_ps[:, c0:], v_sb[:, j, D * g:D * (g + 1)], e_t[:, c0:],
                                start=(j == 0), stop=(j == ntk - 1),
                            )
                            nc.tensor.matmul(
                                s_row[:, c0:], ones_sb[:], e_t[:, c0:],
                                start=(j == 0), stop=(j == ntk - 1),
                            )
                        srow_sb = np_.tile([1, CH], f32, name="srow_sb", tag="srow_sb")
                        nc.any.tensor_copy(out=srow_sb[:], in_=s_row[:])
                        rec = np_.tile([1, CH], f32, name="rec", tag="rec")
                        nc.vector.reciprocal(rec[:], srow_sb[:])
                        bc = np_.tile([P, CH], f32, name="bc", tag="bc")
                        nc.gpsimd.partition_broadcast(bc[:], rec[:])
                        nc.vector.tensor_mul(qT_sb[h][:, sl], o_ps[:], bc[:])

                    for g in range(NKV):
                        proj_qk(wk_sb, g, kT_sb[g])

                    for u in range(8):
                        psv = pj.tile([P, CH], f32, name="psv", tag="pj")
                        for k2 in range(2):
                            tb = 2 * u + k2
                            for eo in range(EO):
                                nc.tensor.matmul(
                                    psv[:, 256 * k2:256 * (k2 + 1)],
                                    xT_sb[:, eo, P * tb:P * (tb + 1)],
                                    wv_sb[:, eo, :],
                                    start=(eo == 0), stop=(eo == EO - 1),
                                )
                        for k2 in range(2):
                            nc.any.tensor_copy(out=v_sb[:, 2 * u + k2, :], in_=psv[:, 256 * k2:256 * (k2 + 1)])

                    for h in range(NH):
                        proj_qk(wq_sb, h, qT_sb[h])

                    if _PHASE_LIMIT == "proj":
                        for h in range(NH):
                            nc.sync.dma_start(o_r[:, 4 * h, :], qT_sb[h][:])
                        for g in range(NKV):
                            nc.sync.dma_start(o_r[:, 8 + g, :], kT_sb[g][:])
                        return

                    # chunk-major attention, with the partial output projection for each
                    # chunk's t-blocks interleaved one head-slot later (so the softmax
                    # normalize chain has drained), PSUM accumulators reusing the idle
                    # projection-pool slots, copies alternating ACT/DVE, DMAs per t-block.
                    def wo_group(tci):
                        for tb in range(4 * tci, 4 * tci + 4):
                            ost = op_.tile([P, E], bf16, name="ost", tag="ost")
                            for n in range(4):
                                wop = pj.tile([P, CH], f32, name="wop", tag="pj")
                                for h in range(NH):
                                    nc.tensor.matmul(
                                        wop[:],
                                        qT_sb[h][:, P * tb:P * (tb + 1)],
                                        wo_sb[:, h, CH * n:CH * (n + 1)],
                                        start=(h == 0), stop=(h == NH - 1),
                                    )
                                if n % 2 == 0:
                                    nc.scalar.activation(
                                        ost[:, CH * n:CH * (n + 1)], wop[:],
                                        mybir.ActivationFunctionType.Copy,
                                    )
                                else:
                                    nc.vector.tensor_copy(out=ost[:, CH * n:CH * (n + 1)], in_=wop[:])
                            nc.sync.dma_start(o_r[:, tb, :], ost[:])

                    if _PHASE_LIMIT == "attn":
                        for tci in range(NTQ):
                            for h in range(NH):
                                attn(h, tci)
                        for h in range(NH):
                            nc.sync.dma_start(o_r[:, 4 * h, :], qT_sb[h][:])
                        return

                    for tci in range(NTQ):
                        for h in range(NH):
                            attn(h, tci)
                            if h == 0 and tci > 0:
                                wo_group(tci - 1)
                    wo_group(NTQ - 1)

            if reps > 1:
                with tc.For_i(0, reps, 1):
                    emit_body()
            else:
                emit_body()

    nc.finalize()
    return nc


def get_nc(reps=1):
    if reps not in _NC_CACHE:
        _NC_CACHE[reps] = _build_nc(reps)
    return _NC_CACHE[reps]


def make_host_inputs(x, wq, wk, wv, wo):
    """Returns per-core in_maps (list of 8 dicts)."""
    perm = np.concatenate([np.arange(0, D, 2), np.arange(1, D, 2)])
    wq4 = np.asarray(wq).reshape(E, H, D)[:, :, perm]
    wk4 = np.asarray(wk).reshape(E, KVH, D)[:, :, perm]
    wv4 = np.asarray(wv).reshape(E, KVH, D)
    wo4 = np.asarray(wo).reshape(H, D, E)
    xT = np.ascontiguousarray(np.transpose(np.asarray(x), (0, 2, 1))).astype(BF16)

    # mirror reference's float32 rope computation
    invf = 1.0 / (np.float32(THETA) ** (np.arange(0, D, 2, dtype=np.float32) / np.float32(D)))
    ang = np.arange(T, dtype=np.float32)[None, :] * invf[:, None]     # [64, T]
    cosv = np.cos(ang).astype(np.float32)
    sinv = np.sin(ang).astype(np.float32)
    cos_h = np.concatenate([cosv, cosv], 0)
    sin_h = np.concatenate([-sinv, sinv], 0)

    ii = np.arange(P)[:, None]
    jj = np.arange(CH)[None, :]
    mk_h = np.stack([(jj >= ii + P * di) for di in range(4)]).astype(BF16)

    in_maps = []
    for c in range(8):
        b, hg = divmod(c, 4)
        qs = slice(4 * hg, 4 * hg + 4)
        ks = slice(2 * hg, 2 * hg + 2)
        in_maps.append({
            "xT": xT[b],
            "wq": np.ascontiguousarray(wq4[:, qs].reshape(E, NH * D)).astype(BF16),
            "wk": np.ascontiguousarray(wk4[:, ks].reshape(E, NKV * D)).astype(BF16),
            "wv": np.ascontiguousarray(wv4[:, ks].reshape(E, NKV * D)).astype(BF16),
            "wo": np.ascontiguousarray(wo4[qs].reshape(NH * D, E)).astype(BF16),
            "cosd": cos_h,
            "sind": sin_h,
            "mkd": mk_h,
        })
    return in_maps


def kernel(x, mask, wq, wk, wv, wo, **extra):
    from concourse.bass_utils import run_bass_kernel_spmd

    nc = get_nc()
    in_maps = make_host_inputs(x, wq, wk, wv, wo)
    res = run_bass_kernel_spmd(nc, in_maps, core_ids=list(range(8)))
    out = np.zeros((B, T, E), np.float32)
    for c in range(8):
        out[c // 4] += res.results[c]["od"].astype(np.float32)
    return out


# revision 20
# speedup vs baseline: 1.1357x; 1.1357x over previous
"""Trainium2 Bass kernel for causal GQA attention (B=2, T=2048, E=2048, H=16, D=128, KVH=8).

Sharding: 8 cores = 2 (batch) x 4 (head groups). Each core computes 4 query heads
(column-parallel wq) + their 2 KV heads, full causal attention for those heads, and
a partial output projection (row-parallel wo). Host sums the 4 partials per batch.

Layout strategy: everything head-transposed ([D, T] with D on partitions) so that
no on-chip transposes are needed anywhere:
  - qT/kT = wq/wk.T @ x.T directly from PE (lhsT = weight slice, rhs = x.T)
  - scores S^T[k, q] = kT.T @ qT (lhsT = kT block, rhs = qT chunk)
  - attn_outT[d, q] = v_nat.T @ expS^T (lhsT = v natural [t, d], rhs = exp block)
  - out[t, e] = attn_outT.T @ wo (lhsT = attn_outT, rhs = wo rows)
RoPE pairs are de-interleaved by permuting wq/wk columns on the host (scores are
invariant since q and k use the same permutation), so rope becomes a half-swap.
Softmax is computed without max-subtraction (logits are O(5)); causal masking is a
0/1 multiply on the exp'd diagonal blocks, full blocks above the diagonal skipped.
Denominators via ones-vector matmuls accumulated in PSUM alongside the AV matmuls.

`reps`: wraps the whole body (including input DMA) in an on-device For_i loop —
used only for latency-slope timing in test.py; the graded path uses reps=1.
"""

import numpy as np
import ml_dtypes

BF16 = ml_dtypes.bfloat16

B, T, E = 2, 2048, 2048
H, D = 16, 128
KVH = 8
THETA = 10000.0
P = 128
EO = E // P          # 16 contraction chunks
CH = 512             # q-chunk width
NTQ = T // CH        # 4 q chunks
NTB = T // P         # 16 t blocks
NH = H // 4          # 4 q heads per core
NKV = 2              # kv heads per core
SCALE = float(D) ** -0.5

_NC_CACHE = {}
_PHASE_LIMIT = "full"   # "proj" | "attn" | "full" — for phase-cost probing only


def _build_nc(reps=1):
    import concourse.mybir as mybir
    import concourse.tile as tile
    from concourse import bacc

    nc = bacc.Bacc(None, target_bir_lowering=False)
    dt = mybir.dt
    f32, bf16 = dt.float32, dt.bfloat16
    Exp = mybir.ActivationFunctionType.Exp

    xT_d = nc.dram_tensor("xT", [E, T], bf16, kind="ExternalInput")
    wq_d = nc.dram_tensor("wq", [E, NH * D], bf16, kind="ExternalInput")
    wk_d = nc.dram_tensor("wk", [E, NKV * D], bf16, kind="ExternalInput")
    wv_d = nc.dram_tensor("wv", [E, NKV * D], bf16, kind="ExternalInput")
    wo_d = nc.dram_tensor("wo", [NH * D, E], bf16, kind="ExternalInput")
    cos_d = nc.dram_tensor("cosd", [P, T], f32, kind="ExternalInput")
    sin_d = nc.dram_tensor("sind", [P, T], f32, kind="ExternalInput")
    mk_d = nc.dram_tensor("mkd", [4, P, CH], bf16, kind="ExternalInput")
    o_d = nc.dram_tensor("od", [T, E], bf16, kind="ExternalOutput")

    xT_r = xT_d.rearrange("(eo p) t -> p eo t", p=P)
    wq_r = wq_d.rearrange("(eo p) m -> p eo m", p=P)
    wk_r = wk_d.rearrange("(eo p) m -> p eo m", p=P)
    wv_r = wv_d.rearrange("(eo p) m -> p eo m", p=P)
    wo_r = wo_d.rearrange("(h p) e -> p h e", p=P)
    o_r = o_d.rearrange("(tb p) e -> p tb e", p=P)

    with tile.TileContext(nc) as tc:
        with (
            tc.tile_pool(name="singles", bufs=1) as sg,
            tc.tile_pool(name="ropet", bufs=2) as rp,
            tc.tile_pool(name="expp", bufs=8) as ep,
            tc.tile_pool(name="normp", bufs=2) as np_,
            tc.tile_pool(name="outst", bufs=2) as op_,
        ):

            def emit_body():
                # tiles (allocation only; DMA issue order below is what matters)
                wk_sb = sg.tile([P, EO, NKV * D], bf16, name="wk_sb", tag="wk_sb")
                xT_sb = sg.tile([P, EO, T], bf16, name="xT_sb", tag="xT_sb")
                wv_sb = sg.tile([P, EO, NKV * D], bf16, name="wv_sb", tag="wv_sb")
                wq_sb = sg.tile([P, EO, NH * D], bf16, name="wq_sb", tag="wq_sb")
                cos_sb = sg.tile([P, T], f32, name="cos_sb", tag="cos_sb")
                sin_sb = sg.tile([P, T], f32, name="sin_sb", tag="sin_sb")
                wo_sb = sg.tile([P, NH, E], bf16, name="wo_sb", tag="wo_sb")
                mk_sb = [sg.tile([P, CH], bf16, name=f"mk{i}", tag=f"mk{i}") for i in range(4)]

                # DMA issue order = consumption order
                nc.sync.dma_start(wk_sb[:], wk_r[:])
                for eo in range(EO):
                    nc.sync.dma_start(xT_sb[:, eo, :], xT_r[:, eo, :])
                nc.sync.dma_start(cos_sb[:], cos_d[:])
                nc.sync.dma_start(sin_sb[:], sin_d[:])
                nc.sync.dma_start(wv_sb[:], wv_r[:])
                nc.sync.dma_start(wq_sb[:], wq_r[:])
                for i in range(4):
                    nc.sync.dma_start(mk_sb[i][:], mk_d[i])
                nc.sync.dma_start(wo_sb[:], wo_r[:])

                ones_sb = sg.tile([P, 1], bf16, name="ones_sb", tag="ones_sb")
                nc.vector.memset(ones_sb[:], 1.0)

                kT_sb = [sg.tile([P, T], bf16, name=f"kT{g}", tag=f"kT{g}") for g in range(NKV)]
                qT_sb = [sg.tile([P, T], bf16, name=f"qT{h}", tag=f"qT{h}") for h in range(NH)]
                v_sb = sg.tile([P, NTB, NKV * D], bf16, name="v_sb", tag="v_sb")

                def rope_chunk(dest, sl, ps):
                    # dest[:, sl] = ps * cos + swap_halves(ps) * sin (sin rows 0:64 pre-negated)
                    t1 = rp.tile([P, CH], f32, name="ropet1", tag="ropet1")
                    nc.vector.tensor_mul(t1[:], ps[:], cos_sb[:, sl])
                    t2 = rp.tile([P, CH], f32, name="ropet2", tag="ropet2")
                    nc.vector.tensor_mul(t2[0:64, :], ps[64:128, :], sin_sb[0:64, sl])
                    nc.vector.tensor_mul(t2[64:128, :], ps[0:64, :], sin_sb[64:128, sl])
                    nc.vector.tensor_add(dest[:, sl], t1[:], t2[:])

                with (
                    tc.tile_pool(name="pj", bufs=3, space="PSUM") as pj,
                    tc.tile_pool(name="ps_s", bufs=2, space="PSUM") as ps_s,
                    tc.tile_pool(name="ps_o", bufs=2, space="PSUM") as ps_o,
                    tc.tile_pool(name="ps_m", bufs=1, space="PSUM") as ps_m,
                ):
                    def proj_qk(w_sb, col, dest, eo_outer=False):
                        # chunk-major, eo-inner: consecutive matmuls accumulate into the
                        # same PSUM bank (avoids the HW bank-cycling HAM penalty).
                        # eo_outer=True (first unit only): consume xT chunks as the input
                        # DMA delivers them — PE is DMA-starved there anyway.
                        if eo_outer:
                            groups = ((0, 1, 2), (3,))
                        else:
                            groups = ((0,), (1,), (2,), (3,))
                        for chunks in groups:
                            psqs = {tci: pj.tile([P, CH], f32, name=f"psq{tci}", tag="pj")
                                    for tci in chunks}
                            for eo in range(EO):
                                for tci in chunks:
                                    nc.tensor.matmul(
                                        psqs[tci][:],
                                        w_sb[:, eo, D * col:D * (col + 1)],
                                        xT_sb[:, eo, CH * tci:CH * (tci + 1)],
                                        start=(eo == 0), stop=(eo == EO - 1),
                                    )
                            for tci in chunks:
                                rope_chunk(dest, slice(CH * tci, CH * (tci + 1)), psqs[tci])

                    def attn(h, tci):
                        g = h // 2
                        sl = slice(CH * tci, CH * (tci + 1))
                        ntk = 4 * tci + 4
                        o_ps = ps_o.tile([P, CH], f32, name="o_ps", tag="o_ps")
                        s_row = ps_m.tile([1, CH], f32, name="s_row", tag="s_row")
                        for j in range(ntk):
                            # diagonal blocks only need columns >= 128*di (rest fully masked)
                            di = j - 4 * tci
                            c0 = P * di if di > 0 else 0
                            qsl = slice(CH * tci + c0, CH * (tci + 1))
                            s_ps = ps_s.tile([P, CH], f32, name="s_ps", tag="s_ps")
                            nc.tensor.matmul(
                                s_ps[:, c0:], kT_sb[g][:, P * j:P * (j + 1)], qT_sb[h][:, qsl],
                                start=True, stop=True,
                            )
                            e_t = ep.tile([P, CH], bf16, name="e_t", tag="e_t")
                            nc.scalar.activation(e_t[:, c0:], s_ps[:, c0:], Exp, scale=SCALE)
                            if di >= 0:
                                nc.vector.tensor_mul(e_t[:, c0:], e_t[:, c0:], mk_sb[di][:, c0:])
                            nc.tensor.matmul(
                                o_ps[:, c0:], v_sb[:, j, D * g:D * (g + 1)], e_t[:, c0:],
                                start=(j == 0), stop=(j == ntk - 1),
                            )
                            nc.tensor.matmul(
                                s_row[:, c0:], ones_sb[:], e_t[:, c0:],
                                start=(j == 0), stop=(j == ntk - 1),
                            )
                        srow_sb = np_.tile([1, CH], f32, name="srow_sb", tag="srow_sb")
                        nc.any.tensor_copy(out=srow_sb[:], in_=s_row[:])
                        rec = np_.tile([1, CH], f32, name="rec", tag="rec")
                        nc.vector.reciprocal(rec[:], srow_sb[:])
                        bc = np_.tile([P, CH], f32, name="bc", tag="bc")
                        nc.gpsimd.partition_broadcast(bc[:], rec[:])
                        nc.vector.tensor_mul(qT_sb[h][:, sl], o_ps[:], bc[:])

                    for g in range(NKV):
                        proj_qk(wk_sb, g, kT_sb[g])

                    for u in range(8):
                        psv = pj.tile([P, CH], f32, name="psv", tag="pj")
                        for k2 in range(2):
                            tb = 2 * u + k2
                            for eo in range(EO):
                                nc.tensor.matmul(
                                    psv[:, 256 * k2:256 * (k2 + 1)],
                                    xT_sb[:, eo, P * tb:P * (tb + 1)],
                                    wv_sb[:, eo, :],
                                    start=(eo == 0), stop=(eo == EO - 1),
                                )
                        for k2 in range(2):
                            nc.any.tensor_copy(out=v_sb[:, 2 * u + k2, :], in_=psv[:, 256 * k2:256 * (k2 + 1)])

                    for h in range(NH):
                        proj_qk(wq_sb, h, qT_sb[h])

                    if _PHASE_LIMIT == "proj":
                        for h in range(NH):
                            nc.sync.dma_start(o_r[:, 4 * h, :], qT_sb[h][:])
                        for g in range(NKV):
                            nc.sync.dma_start(o_r[:, 8 + g, :], kT_sb[g][:])
                        return

                    # chunk-major attention, with the partial output projection for each
                    # chunk's t-blocks interleaved one head-slot later (so the softmax
                    # normalize chain has drained), PSUM accumulators reusing the idle
                    # projection-pool slots, copies alternating ACT/DVE, DMAs per t-block.
                    def wo_group(tci):
                        for tb in range(4 * tci, 4 * tci + 4):
                            ost = op_.tile([P, E], bf16, name="ost", tag="ost")
                            for n in range(4):
                                wop = pj.tile([P, CH], f32, name="wop", tag="pj")
                                for h in range(NH):
                                    nc.tensor.matmul(
                                        wop[:],
                                        qT_sb[h][:, P * tb:P * (tb + 1)],
                                        wo_sb[:, h, CH * n:CH * (n + 1)],
                                        start=(h == 0), stop=(h == NH - 1),
                                    )
                                if n % 2 == 0:
                                    nc.scalar.activation(
                                        ost[:, CH * n:CH * (n + 1)], wop[:],
                                        mybir.ActivationFunctionType.Copy,
                                    )
                                else:
                                    nc.vector.tensor_copy(out=ost[:, CH * n:CH * (n + 1)], in_=wop[:])
                            nc.sync.dma_start(o_r[:, tb, :], ost[:])

                    if _PHASE_LIMIT == "attn":
                        for tci in range(NTQ):
                            for h in range(NH):
                                attn(h, tci)
                        for h in range(NH):
                            nc.sync.dma_start(o_r[:, 4 * h, :], qT_sb[h][:])
                        return

                    for tci in range(NTQ):
                        for h in range(NH):
                            attn(h, tci)
                            if h == 0 and tci > 0:
                                wo_group(tci - 1)
                    wo_group(NTQ - 1)

            if reps > 1:
                with tc.For_i(0, reps, 1):
                    emit_body()
            else:
                emit_body()

    nc.finalize()
    return nc


def get_nc(reps=1):
    if reps not in _NC_CACHE:
        _NC_CACHE[reps] = _build_nc(reps)
    return _NC_CACHE[reps]


def make_host_inputs(x, wq, wk, wv, wo):
    """Returns per-core in_maps (list of 8 dicts)."""
    perm = np.concatenate([np.arange(0, D, 2), np.arange(1, D, 2)])
    wq4 = np.asarray(wq).reshape(E, H, D)[:, :, perm]
    wk4 = np.asarray(wk).reshape(E, KVH, D)[:, :, perm]
    wv4 = np.asarray(wv).reshape(E, KVH, D)
    wo4 = np.asarray(wo).reshape(H, D, E)
    xT = np.ascontiguousarray(np.transpose(np.asarray(x), (0, 2, 1))).astype(BF16)

    # mirror reference's float32 rope computation
    invf = 1.0 / (np.float32(THETA) ** (np.arange(0, D, 2, dtype=np.float32) / np.float32(D)))
    ang = np.arange(T, dtype=np.float32)[None, :] * invf[:, None]     # [64, T]
    cosv = np.cos(ang).astype(np.float32)
    sinv = np.sin(ang).astype(np.float32)
    cos_h = np.concatenate([cosv, cosv], 0)
    sin_h = np.concatenate([-sinv, sinv], 0)

    ii = np.arange(P)[:, None]
    jj = np.arange(CH)[None, :]
    mk_h = np.stack([(jj >= ii + P * di) for di in range(4)]).astype(BF16)

    in_maps = []
    for c in range(8):
        b, hg = divmod(c, 4)
        qs = slice(4 * hg, 4 * hg + 4)
        ks = slice(2 * hg, 2 * hg + 2)
        in_maps.append({
            "xT": xT[b],
            "wq": np.ascontiguousarray(wq4[:, qs].reshape(E, NH * D)).astype(BF16),
            "wk": np.ascontiguousarray(wk4[:, ks].reshape(E, NKV * D)).astype(BF16),
            "wv": np.ascontiguousarray(wv4[:, ks].reshape(E, NKV * D)).astype(BF16),
            "wo": np.ascontiguousarray(wo4[qs].reshape(NH * D, E)).astype(BF16),
            "cosd": cos_h,
            "sind": sin_h,
            "mkd": mk_h,
        })
    return in_maps


def kernel(x, mask, wq, wk, wv, wo, **extra):
    from concourse.bass_utils import run_bass_kernel_spmd

    nc = get_nc()
    in_maps = make_host_inputs(x, wq, wk, wv, wo)
    res = run_bass_kernel_spmd(nc, in_maps, core_ids=list(range(8)))
    out = np.zeros((B, T, E), np.float32)
    for c in range(8):
        out[c // 4] += res.results[c]["od"].astype(np.float32)
    return out
